# revision 25
# baseline (speedup 1.0000x reference)
"""DLSTMCell hypernetwork kernel for Trainium2 (runs on 4 of 8 NeuronCores).

Reference computation (per stock n of N=2048):
    mem  = emb_table[index]                       (N, 128)
    h1   = tanh(mem @ w1.T + b1)                  (N, 128)
    h    = tanh(h1 @ w2.T + b2)                   (N, 64)
    W_n  = (h @ w3.T + b3).reshape(192, 512)      per-stock LSTM weights
    z    = data_n @ W_n + lstm_bias               data = [x, hx]  (192,)
    g    = sigmoid(z); i,f,gg,o = split(g)
    cy   = cx*sigmoid(f) + sigmoid(i)*tanh(gg)
    hy   = sigmoid(o)*tanh(cy)

Key fusion: the (N, 192, 512) = 805MB weights tensor is never materialized.
    z[n,k] = sum_{d,b} (data[n,d]*h[n,b]) * W3perm[(d,b),k] + sum_d B3r[d,k]*data[n,d]
is a standard dense matmul with the SHARED (12288, 512) matrix W3perm against
per-stock outer-product tiles opT[(d,b), n], accumulated in PSUM.

Sharding: data-parallel over stocks on 4 cores (512 each). The 8 visible
cores oversubscribe the device (>4 concurrent cores measured ~2x slower
per core; an 8-core attempt also wedged the device). W3perm is replicated.

v3 layout (all fp8e4m3 on the contraction):
  - w3p is RESIDENT in SBUF (48KB/partition) — loaded once pre-loop, no
    per-iteration weight streaming.
  - The replicated-data factor A of the outer product (A[p,n] = data row
    d(p), 64x partition-replicated) is pre-replicated on the HOST into a
    6.3MB fp8 DRAM array and DMA'd per K-tile-pair unit — this removes the
    PE broadcast matmuls and PSUM staging entirely.
  - op2[u] = A8[u] * hT2 runs on DVE for some units and gpsimd (Pool) for
    the rest (POOL_MOD of every 12) — the elementwise multiply is the
    bottleneck and neither engine alone keeps up with the fp8 PE.
  - Gate matmuls: fp8 DoubleRow, 2 K-tiles per instruction (0.5 cyc/row).
  - Scales: A8 = 32*data (exact power of 2), w3p = 64*w3; sigmoid applies
    1/2048 descale. End-to-end rel err ~1e-4 (z-term is small vs lstm_bias;
    e4m3 noise is compressed by two sigmoids).

Layout: gates kept transposed [k, n] so the gate unit k sits on partitions:
lstm_bias folds into the ACT sigmoid as a per-partition bias and the LSTM
epilogue runs on [128, n] tiles.
"""
import sys

sys.path.insert(0, "/opt/trn_rl_repo")
import numpy as np
import ml_dtypes
import concourse.bacc as bacc
import concourse.mybir as mybir
import concourse.tile as tile
from concourse.bass_utils import run_bass_kernel_spmd

F32 = mybir.dt.float32
F32R = mybir.dt.float32r
BF16 = mybir.dt.bfloat16
FP8 = mybir.dt.float8e4
AF = mybir.ActivationFunctionType

OP_SCALE = 32.0             # folded into dA8 on host (power of 2: exact)
W3_SCALE = 64.0             # keeps w3 (~0.02 sigma) in e4m3 normal range
Z_DESCALE = 1.0 / (OP_SCALE * W3_SCALE)
POOL_MOD = 7                # units with u%12 < POOL_MOD multiply on Pool

N = 2048
INPUT = 64
EMB = 128
BOT = 64
HID = 128
WDIM = 4 * (INPUT + HID) * HID
NCORES = 4
NC_N = N // NCORES          # 512 stocks per core
D = INPUT + HID             # 192
K4 = 4 * HID                # 512 gate columns
KT = (D * BOT) // 128       # 96 contraction K-tiles of 128
KU = KT // 2                # 48 paired K-tiles

_cache = {}


def _build_program(repeat=1):
    """repeat>1 wraps the compute body in a hardware loop — used only for
    wall-clock slope timing (exec_ns ~= (wall[R2]-wall[R1])/(R2-R1))."""
    nc = bacc.Bacc(None)

    di = lambda name, shape, dt: nc.dram_tensor(name, shape, dt, kind="ExternalInput")
    memT_d = di("memT", [EMB, NC_N], F32R)
    dT0_d = di("dT0", [INPUT, NC_N], F32R)
    dT1_d = di("dT1", [HID, NC_N], F32R)
    cxT_d = di("cxT", [HID, NC_N], F32)
    w1T_d = di("w1T", [EMB, EMB], F32R)
    w2T_d = di("w2T", [EMB, BOT], F32R)
    b1_d = di("b1c", [EMB, 1], F32)
    b2_d = di("b2c", [BOT, 1], F32)
    # resident gate weights: w3sb[r, u, j, m] = 64*W3perm[(2u+j)*128+r, m]
    w3p_d = di("w3p", [128, KU, 2, K4], FP8)
    # host-replicated outer-product factor:
    # dA8[p, u, a, c] = 32*q8(data[4u+2a+p//64, c])
    dA8_d = di("dA8", [128, KU, 2, NC_N], FP8)
    b3a_d = di("b3a", [INPUT, K4], F32R)
    b3b_d = di("b3b", [HID, K4], F32R)
    lb_d = di("lbias", [HID, 4], F32)
    hyT_o = nc.dram_tensor("hyT", [HID, NC_N], F32, kind="ExternalOutput")
    cyT_o = nc.dram_tensor("cyT", [HID, NC_N], F32, kind="ExternalOutput")

    with tile.TileContext(nc) as tc:
        with tc.tile_pool(name="const", bufs=1) as const, \
             tc.tile_pool(name="a8", bufs=6) as a8pool, \
             tc.tile_pool(name="op", bufs=14) as opool, \
             tc.tile_pool(name="ep", bufs=2) as ep, \
             tc.tile_pool(name="psG", bufs=1, space="PSUM") as psG, \
             tc.tile_pool(name="psM", bufs=2, space="PSUM") as psM:

            def load(dram, shape, dt, tag=None):
                nm = tag or dram.name
                t = const.tile(shape, dt, tag=nm, name=nm)
                nc.sync.dma_start(t[:], dram[:])
                return t

            memT = load(memT_d, [EMB, NC_N], F32R)
            dT0 = load(dT0_d, [INPUT, NC_N], F32R)
            dT1 = load(dT1_d, [HID, NC_N], F32R)
            cxT = load(cxT_d, [HID, NC_N], F32)
            w1T = load(w1T_d, [EMB, EMB], F32R)
            w2T = load(w2T_d, [EMB, BOT], F32R)
            b1c = load(b1_d, [EMB, 1], F32)
            b2c = load(b2_d, [BOT, 1], F32)
            b3a = load(b3a_d, [INPUT, K4], F32R)
            b3b = load(b3b_d, [HID, K4], F32R)
            lb = load(lb_d, [HID, 4], F32)
            # resident weights: one big pre-loop DMA, split across queues
            w3sb = const.tile([128, KU, 2, K4], FP8, tag="w3sb", name="w3sb")
            for q in range(8):
                nc.sync.dma_start(
                    w3sb[:, q * (KU // 8):(q + 1) * (KU // 8), :, :],
                    w3p_d[:, q * (KU // 8):(q + 1) * (KU // 8), :, :],
                )

            from contextlib import ExitStack
            loop_ctx = ExitStack()
            UN = 8 if repeat > 1 else 1
            if repeat > 1:
                assert repeat % UN == 0, f"repeat {repeat} not divisible by {UN}"
                loop_ctx.enter_context(
                    tc.For_i(0, repeat // UN, 1,
                             hint_engines=(mybir.EngineType.PE,))
                )

            def body():
                # gate accumulators [k-chunk, n] — 4 full PSUM banks
                psg = [
                    psG.tile([128, NC_N], F32, tag=f"g{kc}", name=f"psg{kc}")
                    for kc in range(4)
                ]

                # hypernetwork MLP on its own double-buffered PSUM banks so
                # iteration i+1's MLP overlaps iteration i's gate drain
                m1 = psM.tile([128, NC_N], F32, tag="m1", name="m1")
                m2 = psM.tile([128, NC_N], F32, tag="m2", name="m2")
                nc.tensor.matmul(m1[:], w1T[:], memT[:], start=True, stop=True)
                h1T = ep.tile([128, NC_N], F32R, tag="h1T")
                nc.scalar.activation(h1T[:], m1[:], AF.Tanh, bias=b1c[:])
                nc.tensor.matmul(m2[0:BOT, :], w2T[:], h1T[:],
                                 start=True, stop=True)
                hT2 = ep.tile([128, NC_N], F32R, tag="hT2")
                nc.scalar.activation(hT2[0:BOT, :], m2[0:BOT, :], AF.Tanh,
                                     bias=b2c[:])
                nc.scalar.activation(hT2[BOT:128, :], m2[0:BOT, :], AF.Tanh,
                                     bias=b2c[:])

                # fold the b3 term in first (start=True resets the banks)
                for kc in range(4):
                    ks = slice(kc * 128, kc * 128 + 128)
                    nc.tensor.matmul(psg[kc][:], b3a[:, ks], dT0[:],
                                     start=True, stop=False)
                    nc.tensor.matmul(psg[kc][:], b3b[:, ks], dT1[:],
                                     start=False, stop=False)

                # main contraction: 48 pair-units u = K-tiles (2u, 2u+1).
                # A-factor DMAs are batched CH units per dma_start (the
                # transfer serializes on the issuing sequencer); muls (DVE
                # or Pool, finely interleaved) run LA_MUL units ahead of
                # the PE gate matmuls.
                CH = 4
                NCH = KU // CH
                LA_MUL = 12
                a8_q, op_q = [], []

                def emit_chunk(c):
                    a8c = a8pool.tile([128, CH, 2, NC_N], FP8, tag="a8",
                                      name="a8")
                    nc.sync.dma_start(a8c[:], dA8_d[:, c * CH:(c + 1) * CH, :, :])
                    a8_q.append(a8c)

                def ensure_chunk(c):
                    while len(a8_q) <= c:
                        emit_chunk(len(a8_q))

                def emit_mul(u):
                    c, i = divmod(u, CH)
                    op2 = opool.tile([128, 2, NC_N], FP8, tag="opT", name="op2")
                    eng = nc.gpsimd if (u * 7) % 12 < POOL_MOD else nc.vector
                    eng.tensor_mul(
                        op2[:], a8_q[c][:, i, :, :],
                        hT2[:, None, :].broadcast_to([128, 2, NC_N]),
                    )
                    op_q.append(op2)

                ensure_chunk(1)
                for u in range(min(LA_MUL, KU)):
                    ensure_chunk(min(u // CH + 1, NCH - 1))
                    emit_mul(u)
                for u in range(KU):
                    nxt = u + LA_MUL
                    if nxt < KU:
                        ensure_chunk(min(nxt // CH + 1, NCH - 1))
                        emit_mul(nxt)
                    last = u == KU - 1
                    for kc in range(4):
                        nc.tensor.matmul(
                            psg[kc][:],
                            w3sb[:, u, :, kc * 128:kc * 128 + 128],
                            op_q[u][:],
                            start=False, stop=last,
                            perf_mode=mybir.MatmulPerfMode.DoubleRow,
                        )
                    op_q[u] = None

                # LSTM epilogue on [hid, n] tiles; k-chunk order: i, f, g, o
                g = []
                for kc in range(4):
                    gt = ep.tile([128, NC_N], F32, tag=f"gs{kc}", name=f"gs{kc}")
                    nc.scalar.activation(gt[:], psg[kc][:], AF.Sigmoid,
                                         bias=lb[:, kc:kc + 1], scale=Z_DESCALE)
                    g.append(gt)
                i_t = ep.tile([128, NC_N], F32, tag="i_t")
                nc.scalar.activation(i_t[:], g[0][:], AF.Sigmoid)
                f_t = ep.tile([128, NC_N], F32, tag="f_t")
                nc.scalar.activation(f_t[:], g[1][:], AF.Sigmoid)
                g_t = ep.tile([128, NC_N], F32, tag="g_t")
                nc.scalar.activation(g_t[:], g[2][:], AF.Tanh)
                o_t = ep.tile([128, NC_N], F32, tag="o_t")
                nc.scalar.activation(o_t[:], g[3][:], AF.Sigmoid)

                t1 = ep.tile([128, NC_N], F32, tag="t1")
                nc.vector.tensor_mul(t1[:], cxT[:], f_t[:])
                t2 = ep.tile([128, NC_N], F32, tag="t2")
                nc.vector.tensor_mul(t2[:], i_t[:], g_t[:])
                cy = ep.tile([128, NC_N], F32, tag="cy")
                nc.vector.tensor_add(cy[:], t1[:], t2[:])
                tcy = ep.tile([128, NC_N], F32, tag="tcy")
                nc.scalar.activation(tcy[:], cy[:], AF.Tanh)
                hy = ep.tile([128, NC_N], F32, tag="hy")
                nc.vector.tensor_mul(hy[:], o_t[:], tcy[:])

                nc.scalar.dma_start(cyT_o[:], cy[:])
                nc.scalar.dma_start(hyT_o[:], hy[:])

            for _ in range(UN):
                body()

            loop_ctx.close()

    nc.finalize()
    return nc


def kernel(x, index, hx, cx, emb_table, w1, b1, w2, b2, w3, b3, lstm_bias,
           _trace=False):
    x = np.asarray(x, np.float32)
    index = np.asarray(index)
    hx = np.asarray(hx, np.float32)
    cx = np.asarray(cx, np.float32)
    emb_table = np.asarray(emb_table, np.float32)
    w1 = np.asarray(w1, np.float32)
    b1 = np.asarray(b1, np.float32)
    w2 = np.asarray(w2, np.float32)
    b2 = np.asarray(b2, np.float32)
    w3 = np.asarray(w3, np.float32)
    b3 = np.asarray(b3, np.float32)
    lstm_bias = np.asarray(lstm_bias, np.float32)

    if "nc" not in _cache:
        _cache["nc"] = _build_program()
    nc = _cache["nc"]

    # host-side input prep (sharding + layout)
    mem = emb_table[index]                                   # (N, EMB)
    c = np.ascontiguousarray
    w1T = c(w1.T)
    w2T = c(w2.T)
    b1c = b1.reshape(EMB, 1)
    b2c = b2.reshape(BOT, 1)
    # W3perm[(d*64+b), k] = w3[d*512+k, b]; then pair K-tiles (2u, 2u+1)
    w3perm = w3.reshape(D, K4, BOT).transpose(0, 2, 1).reshape(D * BOT, K4)
    w3pair = w3perm.reshape(KU, 2, 128, K4)                  # [u, j, r, m]
    w3p = c((w3pair * W3_SCALE).transpose(2, 0, 1, 3)).astype(
        ml_dtypes.float8_e4m3)                               # [r, u, j, m]
    zs = OP_SCALE * W3_SCALE
    b3r = b3.reshape(D, K4) * zs
    b3a = c(b3r[0:INPUT])
    b3b = c(b3r[INPUT:D])
    lbias = c(lstm_bias.reshape(4, HID).T)                   # [j, kc]

    # row index of the replicated data factor: rows[p, u, a] = 4u + 2a + p//64
    rows = (np.arange(128)[:, None, None] // 64
            + 4 * np.arange(KU)[None, :, None]
            + 2 * np.arange(2)[None, None, :])               # [128, KU, 2]

    in_maps = []
    for ci in range(NCORES):
        sl = slice(ci * NC_N, (ci + 1) * NC_N)
        dataT = np.concatenate([x[sl].T, hx[sl].T], axis=0)  # (192, NC_N)
        d8 = (dataT * OP_SCALE).astype(ml_dtypes.float8_e4m3)
        dA8 = c(d8[rows])                                    # [128, KU, 2, NC_N]
        in_maps.append({
            "memT": c(mem[sl].T),
            "dT0": c(x[sl].T),
            "dT1": c(hx[sl].T),
            "dA8": dA8,
            "cxT": c(cx[sl].T),
            "w1T": w1T, "w2T": w2T, "b1c": b1c, "b2c": b2c,
            "w3p": w3p, "b3a": b3a, "b3b": b3b,
            "lbias": lbias,
        })

    res = run_bass_kernel_spmd(nc, in_maps, list(range(NCORES)), trace=_trace)
    hy = np.concatenate([r["hyT"].T for r in res.results], axis=0)
    cy = np.concatenate([r["cyT"].T for r in res.results], axis=0)
    if _trace:
        kernel.last_results = res
    return hy.astype(np.float32), cy.astype(np.float32)
